# revision 1
# baseline (speedup 1.0000x reference)
"""CorrelationLoss kernel for 8 TRN2 NeuronCores.

loss = || (1/B) * (X - mean(X))^T (X - mean(X)) - I ||_F   for X [8192, 256].

Sharding: data-parallel over the batch (the memory-roofline-optimal split —
every input element is read exactly once). Each core streams its [1024, 256]
shard through the TensorEngine and produces the partial (uncentered) Gram
matrix  S2_c = X_c^T X_c  — exploiting symmetry it emits only the upper block
row [S2[0:128, 0:256] | S2[128:256, 128:256]].  The 8 per-core outputs are a
partial-sum sharding of the global Gram; the host unshards by summing them
and finishes the tiny [256 x 256] -> scalar tail (mean/centering correction,
subtract identity, Frobenius norm) in numpy - O(W^2) work on 0.25% of the
data, while the 8 MiB streaming work all happens on the NeuronCores.
"""

import numpy as np
from contextlib import ExitStack

B_TOTAL = 8192
W = 256
P = 128
KCH = 8          # 128-row chunks per core shard (1024 / 128)
N_CORES = 8

_CACHED_NC = None
LAST_RESULTS = None  # BassKernelResults of the most recent kernel() call


def _build_nc():
    import concourse.tile as tile
    from concourse import bacc, mybir

    f32 = mybir.dt.float32

    nc = bacc.Bacc(
        "TRN2",
        target_bir_lowering=False,
        debug=False,
        enable_asserts=False,
        num_devices=N_CORES,
    )
    x = nc.dram_tensor("x", [KCH * P, W], f32, kind="ExternalInput").ap()
    out = nc.dram_tensor("S_out", [P, W + P], f32, kind="ExternalOutput").ap()

    with tile.TileContext(nc) as tc, ExitStack() as ctx:
        sb = ctx.enter_context(tc.tile_pool(name="sb", bufs=1))
        ps = ctx.enter_context(tc.tile_pool(name="ps", bufs=1, space="PSUM"))

        # Local shard in SBUF: 8 chunks of [128, 256]
        X = sb.tile([P, KCH * W], f32, tag="X")
        Xv = X[:].rearrange("p (k c) -> p k c", c=W)
        xsrc = x.rearrange("(k p) m -> p k m", p=P)
        for k in range(KCH):
            nc.sync.dma_start(Xv[:, k, :], xsrc[:, k, :])

        # Partial Gram, upper block row only (S2 is symmetric):
        #   ps0 = S2[0:128, 0:256],  ps1 = S2[128:256, 128:256]
        ps0 = ps.tile([P, W], f32, tag="g0")
        ps1 = ps.tile([P, P], f32, tag="g1")
        for k in range(KCH):
            nc.tensor.matmul(
                ps0[:],
                lhsT=Xv[:, k, 0:P],
                rhs=Xv[:, k, :],
                start=(k == 0),
                stop=(k == KCH - 1),
            )
            nc.tensor.matmul(
                ps1[:],
                lhsT=Xv[:, k, P:W],
                rhs=Xv[:, k, P:W],
                start=(k == 0),
                stop=(k == KCH - 1),
            )
        S = sb.tile([P, W + P], f32, tag="S")
        nc.vector.tensor_copy(S[:, 0:W], ps0[:])
        nc.vector.tensor_copy(S[:, W : W + P], ps1[:])
        nc.sync.dma_start(out, S[:])

    nc.compile()
    return nc


def _get_nc():
    global _CACHED_NC
    if _CACHED_NC is None:
        _CACHED_NC = _build_nc()
    return _CACHED_NC


def kernel(embedding, label=None, **_unused):
    import os

    from concourse.bass_utils import run_bass_kernel_spmd

    embedding = np.ascontiguousarray(np.asarray(embedding, dtype=np.float32))
    assert embedding.shape == (B_TOTAL, W), embedding.shape

    nc = _get_nc()
    shard_rows = B_TOTAL // N_CORES
    in_maps = [
        {"x": np.ascontiguousarray(embedding[c * shard_rows : (c + 1) * shard_rows])}
        for c in range(N_CORES)
    ]
    trace = bool(int(os.environ.get("CORR_TRACE", "0")))
    res = run_bass_kernel_spmd(
        nc, in_maps, core_ids=list(range(N_CORES)), trace=trace
    )
    global LAST_RESULTS
    LAST_RESULTS = res

    # Unshard: the per-core outputs are a partial-sum sharding of the global
    # Gram matrix's upper block row — sum them, then finish the O(W^2) tail.
    T = np.zeros((P, W + P), np.float64)
    for c in range(N_CORES):
        T += np.asarray(res.results[c]["S_out"], dtype=np.float64)

    s2_top = T[:, 0:W]            # S2[0:128, 0:256]
    s2_br = T[:, W : W + P]       # S2[128:256, 128:256]
    miu = embedding.astype(np.float64).mean(axis=0)
    eye = np.eye(P)
    # D = S2/B - miu miu^T - I, blockwise; D is symmetric so the skipped
    # lower-left block contributes the same sum as the upper-right one.
    d_top = s2_top / B_TOTAL - np.outer(miu[0:P], miu)
    d_top[:, 0:P] -= eye
    d_br = s2_br / B_TOTAL - np.outer(miu[P:W], miu[P:W]) - eye
    ss = (
        (d_top * d_top).sum()
        + (d_top[:, P:W] * d_top[:, P:W]).sum()
        + (d_br * d_br).sum()
    )
    return np.array(np.sqrt(ss), dtype=np.float32)

